# revision 2
# baseline (speedup 1.0000x reference)
"""Tensor-parallel InternLM attention layer for 8 Trainium2 NeuronCores.

Sharding: 32 heads split 4-per-core (column-parallel QKV, row-parallel
o_proj). Each core computes its 4 heads end-to-end (QKV projection, RoPE,
causal attention, partial o_proj); the host sums the 8 partial outputs and
adds the output bias.

Device layout notes:
- All big matmuls run in float32r (full PE rate at N=512, ~1e-3 rel prec).
- Host pre-transposes X and the weight slices so every DMA is contiguous
  and every matmul contracts over the partition dim without on-chip
  transposes.
- Attention runs in scores^T layout [j, i]: softmax normalization over j
  (partitions) is done with an M=1 ones-matmul on the PE, and the 1/sum
  row is replicated across partitions with a K=1 ones-matmul.
"""

import math
from contextlib import ExitStack

import numpy as np

import concourse.bacc as bacc
import concourse.mybir as mybir
import concourse.tile as tile
from concourse.bass_utils import run_bass_kernel_spmd

F32 = mybir.dt.float32
F32R = mybir.dt.float32r
AF = mybir.ActivationFunctionType

P = 128
S = 2048
D = 4096
HD = 128
H = 32
NCORES = 8
HLOC = H // NCORES          # 4 heads per core
M = HLOC * HD               # 512 local qkv width
NK = D // P                 # 32 contraction tiles
IT_W = 512                  # i-tile width in attention
N_IT = S // IT_W            # 4
N_JT = S // P               # 16
SCALE = 1.0 / math.sqrt(HD)

_CACHE = {}


def _classify_blocks(att):
    """att: (S, S) bool, att[i, j] = attend. Returns per-(it, jt) block kind
    in scores^T layout plus the deduped partial-mask tiles (128 j x 512 i)."""
    blocks = []
    masks = []
    mkey = {}
    for it in range(N_IT):
        row = []
        for jt in range(N_JT):
            sub = att[it * IT_W:(it + 1) * IT_W, jt * P:(jt + 1) * P].T
            if not sub.any():
                row.append((0, -1))
            elif sub.all():
                row.append((1, -1))
            else:
                key = sub.tobytes()
                if key not in mkey:
                    mkey[key] = len(masks)
                    masks.append(np.ascontiguousarray(sub, dtype=np.float32))
                row.append((2, mkey[key]))
        blocks.append(tuple(row))
    return tuple(blocks), masks


def _build(blocks, nmask):
    nc = bacc.Bacc("TRN2", target_bir_lowering=False)
    XT = nc.dram_tensor("XT", [D, S], F32R, kind="ExternalInput")
    WQT = nc.dram_tensor("WQT", [D, M], F32R, kind="ExternalInput")
    WKT = nc.dram_tensor("WKT", [D, M], F32R, kind="ExternalInput")
    WVT = nc.dram_tensor("WVT", [D, M], F32R, kind="ExternalInput")
    WOT = nc.dram_tensor("WOT", [M, D], F32R, kind="ExternalInput")
    BQ = nc.dram_tensor("BQ", [P, HLOC], F32, kind="ExternalInput")
    BK = nc.dram_tensor("BK", [P, HLOC], F32, kind="ExternalInput")
    VBBC = nc.dram_tensor("VBBC", [P, M], F32, kind="ExternalInput")
    COS = nc.dram_tensor("COS", [P, S], F32, kind="ExternalInput")
    SIN = nc.dram_tensor("SIN", [P, S], F32, kind="ExternalInput")
    MASKS = nc.dram_tensor("MASKS", [max(nmask, 1), P, IT_W], F32,
                           kind="ExternalInput")
    ONESK = nc.dram_tensor("ONESK", [P, 1], F32R, kind="ExternalInput")
    ONESM = nc.dram_tensor("ONESM", [1, P], F32R, kind="ExternalInput")
    OUT = nc.dram_tensor("OUT", [S, D], F32, kind="ExternalOutput")

    with tile.TileContext(nc) as tc, \
         nc.allow_low_precision(reason="float32r matmul pipeline"), \
         tc.tile_pool(name="dram", bufs=1, space="DRAM") as dpool:
        QKSP = dpool.tile([2, HLOC, P, S], F32R)
        VSP = dpool.tile([S, M], F32R)
        CTXSP = dpool.tile([HLOC, P, S], F32R)

        # ---------------- stage 1: QKV projections + RoPE ----------------
        with ExitStack() as st1:
            sb1 = st1.enter_context(tc.tile_pool(name="sb1", bufs=1))
            xtp = st1.enter_context(tc.tile_pool(name="xtp", bufs=33))
            wp = st1.enter_context(tc.tile_pool(name="wp", bufs=6))
            prep = st1.enter_context(tc.tile_pool(name="prep", bufs=3))
            trig = st1.enter_context(tc.tile_pool(name="trig", bufs=2))
            ps1 = st1.enter_context(
                tc.tile_pool(name="ps1", bufs=1, space="PSUM"))

            bq_sb = sb1.tile([P, HLOC], F32, tag="bq")
            nc.sync.dma_start(bq_sb[:], BQ[:])
            bk_sb = sb1.tile([P, HLOC], F32, tag="bk")
            nc.sync.dma_start(bk_sb[:], BK[:])
            vb_sb = sb1.tile([P, M], F32, tag="vb")
            nc.sync.dma_start(vb_sb[:], VBBC[:])

            for pair in range(2):          # s-chunk pairs of 1024
                s0 = pair * 1024
                xts = []
                for k in range(NK):
                    t = xtp.tile([P, 1024], F32R, tag="xt")
                    nc.sync.dma_start(t[:], XT[k * P:(k + 1) * P, s0:s0 + 1024])
                    xts.append(t)
                cosx = trig.tile([P, 1024], F32, tag="cos")
                nc.sync.dma_start(cosx[:], COS[:, s0:s0 + 1024])
                sinx = trig.tile([P, 1024], F32, tag="sin")
                nc.sync.dma_start(sinx[:], SIN[:, s0:s0 + 1024])

                for qk, (WT, bias_sb) in enumerate(
                        [(WQT, bq_sb), (WKT, bk_sb)]):
                    pss = [ps1.tile([P, 512], F32, tag=f"pa{i}", name=f"ps_qk{i}")
                           for i in range(8)]
                    for k in range(NK):
                        w = wp.tile([P, M], F32R, tag="w")
                        nc.sync.dma_start(w[:], WT[k * P:(k + 1) * P, :])
                        for m in range(HLOC):
                            for c in range(2):
                                nc.tensor.matmul(
                                    pss[m * 2 + c][:],
                                    w[:, m * P:(m + 1) * P],
                                    xts[k][:, c * 512:(c + 1) * 512],
                                    start=(k == 0), stop=(k == NK - 1))
                    for m in range(HLOC):
                        for c in range(2):
                            pre = prep.tile([P, 512], F32, tag="pre")
                            nc.scalar.activation(
                                pre[:], pss[m * 2 + c][:], AF.Identity,
                                bias=bias_sb[:, m:m + 1])
                            sw = prep.tile([P, 512], F32, tag="sw")
                            nc.sync.dma_start(sw[0:64, :], pre[64:128, :])
                            nc.sync.dma_start(sw[64:128, :], pre[0:64, :])
                            cs = cosx[:, c * 512:(c + 1) * 512]
                            sn = sinx[:, c * 512:(c + 1) * 512]
                            rot = prep.tile([P, 512], F32R, tag="rot")
                            nc.vector.tensor_mul(sw[:], sw[:], sn)
                            nc.vector.tensor_mul(pre[:], pre[:], cs)
                            nc.vector.tensor_add(rot[:], pre[:], sw[:])
                            nc.sync.dma_start(
                                QKSP[qk, m, :,
                                     s0 + c * 512:s0 + (c + 1) * 512],
                                rot[:])
                # V projection (layout [s, m], no rope)
                psv = [ps1.tile([P, 512], F32, tag=f"pa{i}", name=f"ps_v{i}") for i in range(8)]
                for k in range(NK):
                    wv = wp.tile([P, M], F32R, tag="w")
                    nc.sync.dma_start(wv[:], WVT[k * P:(k + 1) * P, :])
                    for ss in range(8):
                        nc.tensor.matmul(
                            psv[ss][:],
                            xts[k][:, ss * P:(ss + 1) * P],
                            wv[:],
                            start=(k == 0), stop=(k == NK - 1))
                for ss in range(8):
                    vo = prep.tile([P, M], F32R, tag="vo")
                    nc.vector.tensor_add(vo[:], psv[ss][:], vb_sb[:])
                    nc.sync.dma_start(
                        VSP[s0 + ss * P:s0 + (ss + 1) * P, :], vo[:])

        # ---------------- stage 2: causal attention ----------------
        with ExitStack() as st2:
            sb2 = st2.enter_context(tc.tile_pool(name="sb2", bufs=1))
            qkp = st2.enter_context(tc.tile_pool(name="qkp", bufs=2))
            expp = st2.enter_context(tc.tile_pool(name="expp", bufs=6))
            smallp = st2.enter_context(tc.tile_pool(name="smallp", bufs=4))
            ps2 = st2.enter_context(
                tc.tile_pool(name="ps2", bufs=1, space="PSUM"))

            mask_sb = []
            for mi in range(nmask):
                mt = sb2.tile([P, IT_W], F32, tag=f"mask{mi}")
                nc.sync.dma_start(mt[:], MASKS[mi])
                mask_sb.append(mt)
            ones_k = sb2.tile([P, 1], F32R, tag="onesk")
            nc.sync.dma_start(ones_k[:], ONESK[:])
            ones_m = sb2.tile([1, P], F32R, tag="onesm")
            nc.sync.dma_start(ones_m[:], ONESM[:])

            vsp_r = VSP[:].rearrange("(jt p) m -> p jt m", p=P)
            for h in range(HLOC):
                qt = qkp.tile([P, S], F32R, tag="qt")
                nc.sync.dma_start(qt[:], QKSP[0, h])
                kt = qkp.tile([P, S], F32R, tag="kt")
                nc.sync.dma_start(kt[:], QKSP[1, h])
                vh = qkp.tile([P, N_JT, P], F32R, tag="vh")
                nc.sync.dma_start(vh[:], vsp_r[:, :, h * P:(h + 1) * P])
                for it in range(N_IT):
                    isl = slice(it * IT_W, (it + 1) * IT_W)
                    j_list = [(jt, blocks[it][jt][1])
                              for jt in range(N_JT) if blocks[it][jt][0] != 0]
                    ps_ctx = ps2.tile([P, IT_W], F32, tag="ctx")
                    ps_sum = ps2.tile([1, IT_W], F32, tag="sum")
                    for idx, (jt, mi) in enumerate(j_list):
                        first = idx == 0
                        last = idx == len(j_list) - 1
                        ps_s = ps2.tile([P, IT_W], F32, tag="sc")
                        nc.tensor.matmul(
                            ps_s[:], kt[:, jt * P:(jt + 1) * P], qt[:, isl],
                            start=True, stop=True)
                        ex = expp.tile([P, IT_W], F32R, tag="ex")
                        nc.scalar.activation(ex[:], ps_s[:], AF.Exp,
                                             scale=SCALE)
                        if mi >= 0:
                            nc.vector.tensor_mul(ex[:], ex[:], mask_sb[mi][:])
                        nc.tensor.matmul(ps_sum[:], ones_k[:], ex[:],
                                         start=first, stop=last)
                        nc.tensor.matmul(ps_ctx[:], vh[:, jt, :], ex[:],
                                         start=first, stop=last)
                    rec = smallp.tile([1, IT_W], F32R, tag="rec")
                    nc.vector.reciprocal(rec[:], ps_sum[:])
                    ps_bc = ps2.tile([P, IT_W], F32, tag="bc")
                    nc.tensor.matmul(ps_bc[:], ones_m[:], rec[:],
                                     start=True, stop=True)
                    bc = expp.tile([P, IT_W], F32, tag="bc")
                    nc.vector.tensor_copy(bc[:], ps_bc[:])
                    cto = expp.tile([P, IT_W], F32R, tag="cto")
                    nc.vector.tensor_mul(cto[:], ps_ctx[:], bc[:])
                    nc.sync.dma_start(CTXSP[h, :, isl], cto[:])

        # ---------------- stage 3: o_proj (row-parallel partial) --------
        with ExitStack() as st3:
            sb3 = st3.enter_context(tc.tile_pool(name="sb3", bufs=1))
            wop = st3.enter_context(tc.tile_pool(name="wop", bufs=2))
            outp = st3.enter_context(tc.tile_pool(name="outp", bufs=4))
            ps3 = st3.enter_context(
                tc.tile_pool(name="ps3", bufs=4, space="PSUM"))

            ctx_sb = []
            for h in range(HLOC):
                ct = sb3.tile([P, S], F32R, tag=f"ctx{h}")
                nc.sync.dma_start(ct[:], CTXSP[h])
                ctx_sb.append(ct)
            wot_r = WOT[:].rearrange("(t p) n -> p t n", p=P)
            for n in range(D // 512):
                nsl = slice(n * 512, (n + 1) * 512)
                wo = wop.tile([P, HLOC, 512], F32R, tag="wo")
                nc.sync.dma_start(wo[:], wot_r[:, :, nsl])
                for st in range(S // P):
                    pso = ps3.tile([P, 512], F32, tag="po")
                    for h in range(HLOC):
                        nc.tensor.matmul(
                            pso[:], ctx_sb[h][:, st * P:(st + 1) * P],
                            wo[:, h, :],
                            start=(h == 0), stop=(h == HLOC - 1))
                    ot = outp.tile([P, 512], F32, tag="ot")
                    nc.vector.tensor_copy(ot[:], pso[:])
                    nc.sync.dma_start(OUT[st * P:(st + 1) * P, nsl], ot[:])
    nc.compile()
    return nc


def _get_nc(blocks, nmask):
    key = (blocks, nmask)
    if key not in _CACHE:
        _CACHE[key] = _build(blocks, nmask)
    return _CACHE[key]


def _rope_tables():
    inv_freq = 1.0 / (10000.0 ** (np.arange(0, HD, 2, dtype=np.float64) / HD))
    t = np.arange(S, dtype=np.float64)
    freqs = np.outer(t, inv_freq)            # (S, 64)
    cos = np.cos(freqs).astype(np.float32)
    sin = np.sin(freqs).astype(np.float32)
    cos2 = np.concatenate([cos.T, cos.T], axis=0)             # (128, S)
    sin2 = np.concatenate([-sin.T, sin.T], axis=0)            # (128, S)
    return np.ascontiguousarray(cos2), np.ascontiguousarray(sin2)


def kernel(hidden_states, Wq, bq, Wk, bk, Wv, bv, Wo, bo, attention_mask):
    X = np.asarray(hidden_states, dtype=np.float32)[0]        # (S, D)
    Wq = np.asarray(Wq, dtype=np.float32)
    Wk = np.asarray(Wk, dtype=np.float32)
    Wv = np.asarray(Wv, dtype=np.float32)
    Wo = np.asarray(Wo, dtype=np.float32)
    bq = np.asarray(bq, dtype=np.float32)
    bk = np.asarray(bk, dtype=np.float32)
    bv = np.asarray(bv, dtype=np.float32)
    bo = np.asarray(bo, dtype=np.float32)
    att = np.asarray(attention_mask)[0, 0]

    blocks, masks = _classify_blocks(att)
    nmask = len(masks)
    masks_arr = (np.stack(masks) if nmask
                 else np.zeros((1, P, IT_W), np.float32))
    nc = _get_nc(blocks, nmask)

    XT = np.ascontiguousarray(X.T)
    cos2, sin2 = _rope_tables()
    onesk = np.ones((P, 1), np.float32)
    onesm = np.ones((1, P), np.float32)

    in_maps = []
    for c in range(NCORES):
        sl = slice(c * M, (c + 1) * M)
        in_maps.append({
            "XT": XT,
            "WQT": np.ascontiguousarray(Wq[sl, :].T),
            "WKT": np.ascontiguousarray(Wk[sl, :].T),
            "WVT": np.ascontiguousarray(Wv[sl, :].T),
            "WOT": np.ascontiguousarray(Wo[:, sl].T),
            "BQ": np.ascontiguousarray(bq[sl].reshape(HLOC, P).T),
            "BK": np.ascontiguousarray(bk[sl].reshape(HLOC, P).T),
            "VBBC": np.ascontiguousarray(
                np.broadcast_to(bv[sl], (P, M))),
            "COS": cos2,
            "SIN": sin2,
            "MASKS": masks_arr,
            "ONESK": onesk,
            "ONESM": onesm,
        })
    res = run_bass_kernel_spmd(nc, in_maps, core_ids=list(range(NCORES)))
    out = res.results[0]["OUT"].astype(np.float64)
    for c in range(1, NCORES):
        out += res.results[c]["OUT"]
    out = (out + bo).astype(np.float32)
    return out[None]


# revision 3
# speedup vs baseline: 1.2727x; 1.2727x over previous
"""Tensor-parallel InternLM attention layer for 8 Trainium2 NeuronCores.

Sharding: 32 heads split 4-per-core (column-parallel QKV, row-parallel
o_proj). Each core computes its 4 heads end-to-end (QKV projection, RoPE,
causal attention, partial o_proj); the host sums the 8 partial outputs and
adds the output bias.

Device layout notes:
- All big matmuls run in float32r (full PE rate at N=512, ~1e-3 rel prec).
- Host pre-transposes X and the weight slices so every DMA is contiguous
  and every matmul contracts over the partition dim without on-chip
  transposes.
- Attention runs in scores^T layout [j, i]: softmax normalization over j
  (partitions) is done with an M=1 ones-matmul on the PE, and the 1/sum
  row is replicated across partitions with a K=1 ones-matmul.
"""

import math
from contextlib import ExitStack

import numpy as np

import concourse.bacc as bacc
import concourse.mybir as mybir
import concourse.tile as tile
from concourse.bass_utils import run_bass_kernel_spmd

F32 = mybir.dt.float32
F32R = mybir.dt.float32r
AF = mybir.ActivationFunctionType

P = 128
S = 2048
D = 4096
HD = 128
H = 32
NCORES = 8
HLOC = H // NCORES          # 4 heads per core
M = HLOC * HD               # 512 local qkv width
NK = D // P                 # 32 contraction tiles
IT_W = 512                  # i-tile width in attention
N_IT = S // IT_W            # 4
N_JT = S // P               # 16
SCALE = 1.0 / math.sqrt(HD)

_CACHE = {}


def _classify_blocks(att):
    """att: (S, S) bool, att[i, j] = attend. Returns per-(it, jt) block kind
    in scores^T layout plus the deduped partial-mask tiles (128 j x 512 i)."""
    blocks = []
    masks = []
    mkey = {}
    for it in range(N_IT):
        row = []
        for jt in range(N_JT):
            sub = att[it * IT_W:(it + 1) * IT_W, jt * P:(jt + 1) * P].T
            if not sub.any():
                row.append((0, -1))
            elif sub.all():
                row.append((1, -1))
            else:
                key = sub.tobytes()
                if key not in mkey:
                    mkey[key] = len(masks)
                    masks.append(np.ascontiguousarray(sub, dtype=np.float32))
                row.append((2, mkey[key]))
        blocks.append(tuple(row))
    return tuple(blocks), masks


def _build(blocks, nmask):
    nc = bacc.Bacc("TRN2", target_bir_lowering=False)
    XT = nc.dram_tensor("XT", [D, S], F32R, kind="ExternalInput")
    WQT = nc.dram_tensor("WQT", [D, M], F32R, kind="ExternalInput")
    WKT = nc.dram_tensor("WKT", [D, M], F32R, kind="ExternalInput")
    WVT = nc.dram_tensor("WVT", [D, M], F32R, kind="ExternalInput")
    WOT = nc.dram_tensor("WOT", [M, D], F32R, kind="ExternalInput")
    BQ = nc.dram_tensor("BQ", [P, HLOC], F32, kind="ExternalInput")
    BK = nc.dram_tensor("BK", [P, HLOC], F32, kind="ExternalInput")
    VBBC = nc.dram_tensor("VBBC", [P, M], F32, kind="ExternalInput")
    COS = nc.dram_tensor("COS", [P, S], F32, kind="ExternalInput")
    SIN = nc.dram_tensor("SIN", [P, S], F32, kind="ExternalInput")
    MASKS = nc.dram_tensor("MASKS", [max(nmask, 1), P, IT_W], F32,
                           kind="ExternalInput")
    ONESK = nc.dram_tensor("ONESK", [P, 1], F32R, kind="ExternalInput")
    ONESM = nc.dram_tensor("ONESM", [1, P], F32R, kind="ExternalInput")
    OUT = nc.dram_tensor("OUT", [S, D], F32, kind="ExternalOutput")

    with tile.TileContext(nc) as tc, \
         nc.allow_low_precision(reason="float32r matmul pipeline"), \
         tc.tile_pool(name="dram", bufs=1, space="DRAM") as dpool:
        QKSP = dpool.tile([2, HLOC, P, S], F32R)
        VSP = dpool.tile([S, M], F32R)
        CTXSP = dpool.tile([HLOC, P, S], F32R)

        # ---------------- stage 1: QKV projections + RoPE ----------------
        with ExitStack() as st1:
            sb1 = st1.enter_context(tc.tile_pool(name="sb1", bufs=1))
            xtp = st1.enter_context(tc.tile_pool(name="xtp", bufs=33))
            wp = st1.enter_context(tc.tile_pool(name="wp", bufs=6))
            prep = st1.enter_context(tc.tile_pool(name="prep", bufs=3))
            trig = st1.enter_context(tc.tile_pool(name="trig", bufs=2))
            ps1 = st1.enter_context(
                tc.tile_pool(name="ps1", bufs=1, space="PSUM"))

            bq_sb = sb1.tile([P, HLOC], F32, tag="bq")
            nc.sync.dma_start(bq_sb[:], BQ[:])
            bk_sb = sb1.tile([P, HLOC], F32, tag="bk")
            nc.sync.dma_start(bk_sb[:], BK[:])
            vb_sb = sb1.tile([P, M], F32, tag="vb")
            nc.sync.dma_start(vb_sb[:], VBBC[:])

            for pair in range(2):          # s-chunk pairs of 1024
                s0 = pair * 1024
                xts = [None] * NK
                for qk, (WT, bias_sb) in enumerate(
                        [(WQT, bq_sb), (WKT, bk_sb)]):
                    pss = [ps1.tile([P, 512], F32, tag=f"pa{i}", name=f"ps_qk{i}")
                           for i in range(8)]
                    for k in range(NK):
                        w = wp.tile([P, M], F32R, tag="w")
                        nc.sync.dma_start(w[:], WT[k * P:(k + 1) * P, :])
                        if qk == 0:
                            t = xtp.tile([P, 1024], F32R, tag="xt",
                                         name=f"xt{k}")
                            nc.sync.dma_start(
                                t[:], XT[k * P:(k + 1) * P, s0:s0 + 1024])
                            xts[k] = t
                        for m in range(HLOC):
                            for c in range(2):
                                nc.tensor.matmul(
                                    pss[m * 2 + c][:],
                                    w[:, m * P:(m + 1) * P],
                                    xts[k][:, c * 512:(c + 1) * 512],
                                    start=(k == 0), stop=(k == NK - 1))
                    if qk == 0:
                        cosx = trig.tile([P, 1024], F32, tag="cos")
                        nc.sync.dma_start(cosx[:], COS[:, s0:s0 + 1024])
                        sinx = trig.tile([P, 1024], F32, tag="sin")
                        nc.sync.dma_start(sinx[:], SIN[:, s0:s0 + 1024])
                    for m in range(HLOC):
                        for c in range(2):
                            pre = prep.tile([P, 512], F32, tag="pre")
                            nc.scalar.activation(
                                pre[:], pss[m * 2 + c][:], AF.Identity,
                                bias=bias_sb[:, m:m + 1])
                            sw = prep.tile([P, 512], F32, tag="sw")
                            nc.sync.dma_start(sw[0:64, :], pre[64:128, :])
                            nc.sync.dma_start(sw[64:128, :], pre[0:64, :])
                            cs = cosx[:, c * 512:(c + 1) * 512]
                            sn = sinx[:, c * 512:(c + 1) * 512]
                            rot = prep.tile([P, 512], F32R, tag="rot")
                            nc.vector.tensor_mul(sw[:], sw[:], sn)
                            nc.vector.tensor_mul(pre[:], pre[:], cs)
                            nc.vector.tensor_add(rot[:], pre[:], sw[:])
                            nc.sync.dma_start(
                                QKSP[qk, m, :,
                                     s0 + c * 512:s0 + (c + 1) * 512],
                                rot[:])
                # V projection (layout [s, m], no rope)
                psv = [ps1.tile([P, 512], F32, tag=f"pa{i}", name=f"ps_v{i}") for i in range(8)]
                for k in range(NK):
                    wv = wp.tile([P, M], F32R, tag="w")
                    nc.sync.dma_start(wv[:], WVT[k * P:(k + 1) * P, :])
                    for ss in range(8):
                        nc.tensor.matmul(
                            psv[ss][:],
                            xts[k][:, ss * P:(ss + 1) * P],
                            wv[:],
                            start=(k == 0), stop=(k == NK - 1))
                for ss in range(8):
                    vo = prep.tile([P, M], F32R, tag="vo")
                    nc.vector.tensor_add(vo[:], psv[ss][:], vb_sb[:])
                    nc.sync.dma_start(
                        VSP[s0 + ss * P:s0 + (ss + 1) * P, :], vo[:])

        # ---------------- stage 2: causal attention ----------------
        with ExitStack() as st2:
            sb2 = st2.enter_context(tc.tile_pool(name="sb2", bufs=1))
            qkp = st2.enter_context(tc.tile_pool(name="qkp", bufs=2))
            expp = st2.enter_context(tc.tile_pool(name="expp", bufs=6))
            smallp = st2.enter_context(tc.tile_pool(name="smallp", bufs=4))
            ps2 = st2.enter_context(
                tc.tile_pool(name="ps2", bufs=1, space="PSUM"))

            mask_sb = []
            for mi in range(nmask):
                mt = sb2.tile([P, IT_W], F32, tag=f"mask{mi}")
                nc.sync.dma_start(mt[:], MASKS[mi])
                mask_sb.append(mt)
            ones_k = sb2.tile([P, 1], F32R, tag="onesk")
            nc.sync.dma_start(ones_k[:], ONESK[:])
            ones_m = sb2.tile([1, P], F32R, tag="onesm")
            nc.sync.dma_start(ones_m[:], ONESM[:])

            vsp_r = VSP[:].rearrange("(jt p) m -> p jt m", p=P)
            for h in range(HLOC):
                qt = qkp.tile([P, S], F32R, tag="qt")
                nc.sync.dma_start(qt[:], QKSP[0, h])
                kt = qkp.tile([P, S], F32R, tag="kt")
                nc.sync.dma_start(kt[:], QKSP[1, h])
                vh = qkp.tile([P, N_JT, P], F32R, tag="vh")
                nc.sync.dma_start(vh[:], vsp_r[:, :, h * P:(h + 1) * P])
                for it in range(N_IT):
                    isl = slice(it * IT_W, (it + 1) * IT_W)
                    j_list = [(jt, blocks[it][jt][1])
                              for jt in range(N_JT) if blocks[it][jt][0] != 0]
                    ps_ctx = ps2.tile([P, IT_W], F32, tag="ctx")
                    ps_sum = ps2.tile([1, IT_W], F32, tag="sum")
                    for idx, (jt, mi) in enumerate(j_list):
                        first = idx == 0
                        last = idx == len(j_list) - 1
                        ps_s = ps2.tile([P, IT_W], F32, tag="sc")
                        nc.tensor.matmul(
                            ps_s[:], kt[:, jt * P:(jt + 1) * P], qt[:, isl],
                            start=True, stop=True)
                        ex = expp.tile([P, IT_W], F32R, tag="ex")
                        nc.scalar.activation(ex[:], ps_s[:], AF.Exp,
                                             scale=SCALE)
                        if mi >= 0:
                            nc.vector.tensor_mul(ex[:], ex[:], mask_sb[mi][:])
                        nc.tensor.matmul(ps_sum[:], ones_k[:], ex[:],
                                         start=first, stop=last)
                        nc.tensor.matmul(ps_ctx[:], vh[:, jt, :], ex[:],
                                         start=first, stop=last)
                    rec = smallp.tile([1, IT_W], F32R, tag="rec")
                    nc.vector.reciprocal(rec[:], ps_sum[:])
                    ps_bc = ps2.tile([P, IT_W], F32, tag="bc")
                    nc.tensor.matmul(ps_bc[:], ones_m[:], rec[:],
                                     start=True, stop=True)
                    bc = expp.tile([P, IT_W], F32, tag="bc")
                    nc.vector.tensor_copy(bc[:], ps_bc[:])
                    cto = expp.tile([P, IT_W], F32R, tag="cto")
                    nc.vector.tensor_mul(cto[:], ps_ctx[:], bc[:])
                    nc.sync.dma_start(CTXSP[h, :, isl], cto[:])

        # ---------------- stage 3: o_proj (row-parallel partial) --------
        with ExitStack() as st3:
            sb3 = st3.enter_context(tc.tile_pool(name="sb3", bufs=1))
            wop = st3.enter_context(tc.tile_pool(name="wop", bufs=3))
            outp = st3.enter_context(tc.tile_pool(name="outp", bufs=6))
            ps3 = st3.enter_context(
                tc.tile_pool(name="ps3", bufs=6, space="PSUM"))

            ctx_sb = []
            for h in range(HLOC):
                ct = sb3.tile([P, S], F32R, tag=f"ctx{h}")
                nc.sync.dma_start(ct[:], CTXSP[h])
                ctx_sb.append(ct)
            wot_r = WOT[:].rearrange("(t p) n -> p t n", p=P)
            for n in range(D // 512):
                nsl = slice(n * 512, (n + 1) * 512)
                wo = wop.tile([P, HLOC, 512], F32R, tag="wo")
                nc.sync.dma_start(wo[:], wot_r[:, :, nsl])
                for st in range(S // P):
                    pso = ps3.tile([P, 512], F32, tag="po")
                    for h in range(HLOC):
                        nc.tensor.matmul(
                            pso[:], ctx_sb[h][:, st * P:(st + 1) * P],
                            wo[:, h, :],
                            start=(h == 0), stop=(h == HLOC - 1))
                    ot = outp.tile([P, 512], F32, tag="ot")
                    nc.vector.tensor_copy(ot[:], pso[:])
                    nc.sync.dma_start(OUT[st * P:(st + 1) * P, nsl], ot[:])
    nc.compile()
    return nc


def _get_nc(blocks, nmask):
    key = (blocks, nmask)
    if key not in _CACHE:
        _CACHE[key] = _build(blocks, nmask)
    return _CACHE[key]


def _rope_tables():
    inv_freq = 1.0 / (10000.0 ** (np.arange(0, HD, 2, dtype=np.float64) / HD))
    t = np.arange(S, dtype=np.float64)
    freqs = np.outer(t, inv_freq)            # (S, 64)
    cos = np.cos(freqs).astype(np.float32)
    sin = np.sin(freqs).astype(np.float32)
    cos2 = np.concatenate([cos.T, cos.T], axis=0)             # (128, S)
    sin2 = np.concatenate([-sin.T, sin.T], axis=0)            # (128, S)
    return np.ascontiguousarray(cos2), np.ascontiguousarray(sin2)


def kernel(hidden_states, Wq, bq, Wk, bk, Wv, bv, Wo, bo, attention_mask):
    X = np.asarray(hidden_states, dtype=np.float32)[0]        # (S, D)
    Wq = np.asarray(Wq, dtype=np.float32)
    Wk = np.asarray(Wk, dtype=np.float32)
    Wv = np.asarray(Wv, dtype=np.float32)
    Wo = np.asarray(Wo, dtype=np.float32)
    bq = np.asarray(bq, dtype=np.float32)
    bk = np.asarray(bk, dtype=np.float32)
    bv = np.asarray(bv, dtype=np.float32)
    bo = np.asarray(bo, dtype=np.float32)
    att = np.asarray(attention_mask)[0, 0]

    blocks, masks = _classify_blocks(att)
    nmask = len(masks)
    masks_arr = (np.stack(masks) if nmask
                 else np.zeros((1, P, IT_W), np.float32))
    nc = _get_nc(blocks, nmask)

    XT = np.ascontiguousarray(X.T)
    cos2, sin2 = _rope_tables()
    onesk = np.ones((P, 1), np.float32)
    onesm = np.ones((1, P), np.float32)

    in_maps = []
    for c in range(NCORES):
        sl = slice(c * M, (c + 1) * M)
        in_maps.append({
            "XT": XT,
            "WQT": np.ascontiguousarray(Wq[sl, :].T),
            "WKT": np.ascontiguousarray(Wk[sl, :].T),
            "WVT": np.ascontiguousarray(Wv[sl, :].T),
            "WOT": np.ascontiguousarray(Wo[:, sl].T),
            "BQ": np.ascontiguousarray(bq[sl].reshape(HLOC, P).T),
            "BK": np.ascontiguousarray(bk[sl].reshape(HLOC, P).T),
            "VBBC": np.ascontiguousarray(
                np.broadcast_to(bv[sl], (P, M))),
            "COS": cos2,
            "SIN": sin2,
            "MASKS": masks_arr,
            "ONESK": onesk,
            "ONESM": onesm,
        })
    res = run_bass_kernel_spmd(nc, in_maps, core_ids=list(range(NCORES)))
    out = res.results[0]["OUT"].astype(np.float64)
    for c in range(1, NCORES):
        out += res.results[c]["OUT"]
    out = (out + bo).astype(np.float32)
    return out[None]
